# revision 10
# baseline (speedup 1.0000x reference)
"""DotGATHead Trainium2 kernel: LN -> 4-head masked attention -> MLP (2x swish+LN+linear) -> LN.

Sharding: query rows. 8 cores = 4 batches x 2 query-halves. Each core gets its
batch's x (rolled so its query rows are always rows 0..1023 -> one SPMD program),
computes K/V over all 2048 nodes, attention + MLP for its 1024 queries.

v2 schedule: P1 (LN+transpose) and P2 (projections) fused into a 512-node chunk
pipeline so attention starts ~40us in; V projection half-deferred to fill PE
stalls during head-0 scores/exp; all 16-bit data in fp16 (10-bit mantissa =
f32r precision for these magnitudes); mask on the scalar DMA ring, x+weights on
the sync ring in consumption order; ACT tables preloaded at t=0; attention
processed in (head, 512-query-chunk) units with a 16->8->4 adder tree + 4-way
accumulating ones-matmul partition reduce; attention output normalized into a
scratch then silu'd straight into s0T (fp16) so the MLP input is ready when
attention ends. LN1/LN2 standardization folded into fc1/fc2 via K=1
augmentation matmuls; final LN via bn_stats on fc2 psum rows.
"""

import numpy as np
import ml_dtypes

import concourse.bass as bass
import concourse.mybir as mybir
import concourse.tile as tile
from concourse import bacc
from concourse.bass_utils import run_bass_kernel_spmd
from concourse.masks import make_identity

B, A, D, HEADS, HD = 4, 2048, 512, 4, 128
P = 128
QLOC = 1024              # query rows per core
NQS = QLOC // P          # 8 query subtiles
NKT = A // P             # 16 key tiles
NEO = D // P             # 4 feature partition-tiles
NQC = QLOC // 512        # 2 query chunks of 512
NCH = A // 512           # 4 node chunks of 512
EPS = 1e-5
F32 = mybir.dt.float32
F32R = mybir.dt.float32r
FP16 = mybir.dt.float16
AF = mybir.ActivationFunctionType
OP = mybir.AluOpType

_compiled = {}


DEBUG_DUMP = False


def _build(use_v_bias, use_qk_bias, use_final_affine):
    nc = bacc.Bacc("TRN2", target_bir_lowering=False, debug=False, num_devices=8)
    dbg = {}
    if DEBUG_DUMP:
        dbg["KT"] = nc.dram_tensor("d_KT", [P, HEADS, A], FP16, kind="ExternalOutput")
        dbg["QT"] = nc.dram_tensor("d_QT", [P, HEADS, QLOC], FP16, kind="ExternalOutput")
        dbg["Vg"] = nc.dram_tensor("d_Vg", [P, NKT, D], FP16, kind="ExternalOutput")
        dbg["maskT"] = nc.dram_tensor("d_maskT", [P, NKT, QLOC], FP16, kind="ExternalOutput")
        dbg["em00"] = nc.dram_tensor("d_em00", [P, NKT, 512], FP16, kind="ExternalOutput")
        dbg["s0T"] = nc.dram_tensor("d_s0T", [P, NEO, QLOC], FP16, kind="ExternalOutput")
        dbg["nmu1"] = nc.dram_tensor("d_nmu1", [P, QLOC], F32, kind="ExternalOutput")
        dbg["sig1"] = nc.dram_tensor("d_sig1", [P, QLOC], F32, kind="ExternalOutput")
        dbg["h1sT"] = nc.dram_tensor("d_h1sT", [P, NEO, QLOC], FP16, kind="ExternalOutput")

    x_d = nc.dram_tensor("x", [A, D], F32, kind="ExternalInput")
    mask_d = nc.dram_tensor("maskT", [P, NKT, QLOC], FP16, kind="ExternalInput")
    wq_d = nc.dram_tensor("wq_t", [D, D], FP16, kind="ExternalInput")
    wk_d = nc.dram_tensor("wk_t", [D, D], FP16, kind="ExternalInput")
    wv_d = nc.dram_tensor("wv_t", [D, D], FP16, kind="ExternalInput")
    fc1_d = nc.dram_tensor("fc1_t", [D, D], FP16, kind="ExternalInput")
    fc2_d = nc.dram_tensor("fc2_t", [D, D], FP16, kind="ExternalInput")
    fc2sum_d = nc.dram_tensor("fc2sum", [1, D], F32R, kind="ExternalInput")
    fc2b_d = nc.dram_tensor("fc2b", [1, D], F32R, kind="ExternalInput")
    qkvb_d = nc.dram_tensor("qkvb", [P, 3 * NEO], F32, kind="ExternalInput")
    naff_d = nc.dram_tensor("naff", [1, 2 * D], F32, kind="ExternalInput")
    vbrow_d = nc.dram_tensor("vbrow", [1, D], F32, kind="ExternalInput")
    onesd_d = nc.dram_tensor("onesd", [P, P], FP16, kind="ExternalInput")
    fc1sum_d = nc.dram_tensor("fc1sum", [1, D], F32R, kind="ExternalInput")
    fc1brow_d = nc.dram_tensor("fc1brow", [1, D], F32R, kind="ExternalInput")
    y_d = nc.dram_tensor("y", [QLOC, D], F32, kind="ExternalOutput")

    with tile.TileContext(nc) as tc:
        with tc.tile_pool(name="const", bufs=1) as const, \
             tc.tile_pool(name="mlpin", bufs=1) as mlpin:
            # --- ACT table preloads (overlap table DMA with input DMA) ---
            dummy = const.tile([P, 1], F32)
            nc.vector.memset(dummy[:], 1.0)
            for fn in (AF.Sqrt, AF.Exp, AF.Silu, AF.Square):
                nc.scalar.activation(out=dummy[:], in_=dummy[:], func=fn)
            nc.scalar.copy(out=dummy[:], in_=dummy[:])

            eps_t = const.tile([P, 1], F32)
            nc.vector.memset(eps_t[:], EPS)
            ident = const.tile([P, P], F32)
            make_identity(nc, ident[:])
            ones_h = const.tile([P, P], FP16)
            nc.vector.memset(ones_h[:], 1.0)
            ones_inv = const.tile([P, P], FP16)
            nc.sync.dma_start(out=ones_inv[:], in_=onesd_d[:, :])
            fc2sum = const.tile([1, D], F32R)
            nc.sync.dma_start(out=fc2sum[:], in_=fc2sum_d[:])
            fc2b = const.tile([1, D], F32R)
            nc.sync.dma_start(out=fc2b[:], in_=fc2b_d[:])
            fc1sum = const.tile([1, D], F32R)
            nc.sync.dma_start(out=fc1sum[:], in_=fc1sum_d[:])
            fc1brow = const.tile([1, D], F32R)
            nc.sync.dma_start(out=fc1brow[:], in_=fc1brow_d[:])
            qkvb = const.tile([P, 3 * NEO], F32)
            if use_qk_bias or use_v_bias:
                nc.sync.dma_start(out=qkvb[:], in_=qkvb_d[:])
            vb_rep = const.tile([P, D], F32)
            if use_v_bias:
                vb_ap = vbrow_d[:, :]
                nc.gpsimd.dma_start(out=vb_rep[:], in_=bass.AP(
                    tensor=vb_ap.tensor, offset=vb_ap.offset,
                    ap=[[0, P], [1, D]]))
            naff = const.tile([P, 2 * D], F32)
            if use_final_affine:
                naff_ap = naff_d[:, :]
                nc.gpsimd.dma_start(out=naff[:], in_=bass.AP(
                    tensor=naff_ap.tensor, offset=naff_ap.offset,
                    ap=[[0, P], [1, 2 * D]]))
            fc1 = const.tile([P, NEO, D], FP16)
            fc2 = const.tile([P, NEO, D], FP16)

            s0T = mlpin.tile([P, NEO, QLOC], FP16)   # silu(attn out), T layout

            # ======== attention-lifetime pools ========
            with tc.tile_pool(name="attw", bufs=1) as attw, \
                 tc.tile_pool(name="expp", bufs=2) as expp, \
                 tc.tile_pool(name="treep", bufs=2) as treep, \
                 tc.tile_pool(name="recp", bufs=2) as recp, \
                 tc.tile_pool(name="scp", bufs=2) as scp, \
                 tc.tile_pool(name="sps", bufs=2, space="PSUM") as sps:
                KT = attw.tile([P, HEADS, A], FP16)       # K^T/sqrt(hd): [d, h, node]
                QT = attw.tile([P, HEADS, QLOC], FP16)    # Q^T: [d, h, q]
                Vg = attw.tile([P, NKT, D], FP16)         # V rows [node, f]
                maskT = attw.tile([P, NKT, QLOC], FP16)
                # mask on the scalar HWDGE ring (parallel with x/weights on sync)
                nc.scalar.dma_start(out=maskT[:], in_=mask_d[:])

                def scores_and_exp(h):
                    """scores^T + exp for head h, both query chunks."""
                    out = []
                    for c in range(NQC):
                        expTm = expp.tile([P, NKT, 512], FP16, tag="expTm")
                        out.append(expTm)
                        for kp in range(NKT // 2):
                            ps = sps.tile([P, 2, 512], F32, tag="sc")
                            for j in range(2):
                                kt = kp * 2 + j
                                nc.tensor.matmul(ps[:, j, :],
                                                 KT[:, h, kt * P:(kt + 1) * P],
                                                 QT[:, h, c * 512:(c + 1) * 512],
                                                 start=True, stop=True)
                            nc.scalar.activation(
                                out=expTm[:, kp * 2:kp * 2 + 2, :],
                                in_=ps[:], func=AF.Exp)
                    return out

                # ---- fused P1+P2: LN chunks -> K/Q projections (+half of V) ----
                with tc.tile_pool(name="xnp", bufs=1) as xnp, \
                     tc.tile_pool(name="p1t", bufs=4) as p1t, \
                     tc.tile_pool(name="p1ps", bufs=1, space="PSUM") as p1ps, \
                     tc.tile_pool(name="p2ps", bufs=2, space="PSUM") as p2ps:
                    xnT = xnp.tile([P, NEO, A], FP16)
                    wq = xnp.tile([P, NEO, D], FP16)
                    wk = xnp.tile([P, NEO, D], FP16)
                    wv = xnp.tile([P, NEO, D], FP16)

                    def v_tile(kt):
                        ps = p2ps.tile([P, D], F32, tag="ps")
                        for eo in range(NEO):
                            nc.tensor.matmul(ps[:], xnT[:, eo, kt * P:(kt + 1) * P],
                                             wv[:, eo, :],
                                             start=(eo == 0), stop=(eo == NEO - 1))
                        if use_v_bias:
                            nc.vector.tensor_tensor(out=ps[:], in0=ps[:],
                                                    in1=vb_rep[:], op=OP.add)
                        nc.scalar.copy(out=Vg[:, kt, :], in_=ps[:])

                    for c in range(NCH):
                        for j in range(4):
                            r = 4 * c + j
                            xt = p1t.tile([P, D], F32, tag="xt")
                            nc.sync.dma_start(out=xt[:], in_=x_d[r * P:(r + 1) * P, :])
                            # weight DMAs interleaved in consumption order.
                            # NOTE: in Tile, emission order IS dataflow order —
                            # every weight DMA must be emitted before its first
                            # consumer or the consumer reads uninitialized SBUF.
                            if r == 1:
                                nc.sync.dma_start(
                                    out=wk[:], in_=wk_d.rearrange("(eo p) f -> p eo f", p=P))
                            elif r == 2:
                                nc.sync.dma_start(
                                    out=wq[:], in_=wq_d.rearrange("(eo p) f -> p eo f", p=P))
                            elif r == 3:
                                nc.sync.dma_start(
                                    out=wv[:], in_=wv_d.rearrange("(eo p) f -> p eo f", p=P))
                            elif r == 9:
                                nc.sync.dma_start(
                                    out=fc1[:], in_=fc1_d.rearrange("(eo p) f -> p eo f", p=P))
                            elif r == 11:
                                nc.sync.dma_start(
                                    out=fc2[:], in_=fc2_d.rearrange("(eo p) f -> p eo f", p=P))
                            # row LayerNorm (standardize only)
                            st6 = p1t.tile([P, 6], F32, tag="st6")
                            nc.vector.bn_stats(out=st6[:], in_=xt[:])
                            mv = p1t.tile([P, 2], F32, tag="mv")
                            nc.vector.bn_aggr(out=mv[:], in_=st6[:])
                            sig = p1t.tile([P, 1], F32, tag="sig")
                            nc.scalar.activation(out=sig[:], in_=mv[:, 1:2], func=AF.Sqrt,
                                                 bias=eps_t[:], scale=1.0)
                            rstd = p1t.tile([P, 1], F32, tag="rstd")
                            nc.vector.reciprocal_approx_fast(out=rstd[:], in_=sig[:])
                            nc.vector.tensor_scalar(out=xt[:], in0=xt[:],
                                                    scalar1=mv[:, 0:1], scalar2=rstd[:],
                                                    op0=OP.subtract, op1=OP.mult)
                            tp = p1ps.tile([P, NEO, P], F32, tag="tp1")
                            for eo in range(NEO):
                                nc.tensor.transpose(tp[:, eo, :], xt[:, eo * P:(eo + 1) * P],
                                                    ident[:])
                            nc.vector.tensor_copy(out=xnT[:, :, r * P:(r + 1) * P], in_=tp[:])
                        # K^T for this node chunk, all heads
                        for h in range(HEADS):
                            ps = p2ps.tile([P, 512], F32, tag="ps")
                            for eo in range(NEO):
                                nc.tensor.matmul(ps[:], wk[:, eo, h * HD:(h + 1) * HD],
                                                 xnT[:, eo, c * 512:(c + 1) * 512],
                                                 start=(eo == 0), stop=(eo == NEO - 1))
                            if use_qk_bias:
                                nc.scalar.activation(out=KT[:, h, c * 512:(c + 1) * 512],
                                                     in_=ps[:], func=AF.Identity,
                                                     bias=qkvb[:, NEO + h:NEO + h + 1],
                                                     scale=1.0)
                            else:
                                nc.scalar.copy(out=KT[:, h, c * 512:(c + 1) * 512], in_=ps[:])
                        # Q^T (queries = rolled nodes 0..QLOC-1)
                        if c < NQC:
                            for h in range(HEADS):
                                ps = p2ps.tile([P, 512], F32, tag="ps")
                                for eo in range(NEO):
                                    nc.tensor.matmul(ps[:], wq[:, eo, h * HD:(h + 1) * HD],
                                                     xnT[:, eo, c * 512:(c + 1) * 512],
                                                     start=(eo == 0), stop=(eo == NEO - 1))
                                if use_qk_bias:
                                    nc.scalar.activation(out=QT[:, h, c * 512:(c + 1) * 512],
                                                         in_=ps[:], func=AF.Identity,
                                                         bias=qkvb[:, h:h + 1], scale=1.0)
                                else:
                                    nc.scalar.copy(out=QT[:, h, c * 512:(c + 1) * 512],
                                                   in_=ps[:])
                        # half of V in-chunk (kt = 4c, 4c+1)
                        v_tile(4 * c)
                        v_tile(4 * c + 1)

                    # head-0 scores+exp next in priority; deferred V fills PE stalls
                    exp_h0 = scores_and_exp(0)
                    for c in range(NCH):
                        v_tile(4 * c + 2)
                        v_tile(4 * c + 3)
                    if DEBUG_DUMP:
                        nc.sync.dma_start(out=dbg["KT"][:, :, :], in_=KT[:])
                        nc.sync.dma_start(out=dbg["QT"][:, :, :], in_=QT[:])
                        nc.sync.dma_start(out=dbg["Vg"][:, :, :], in_=Vg[:])
                        nc.sync.dma_start(out=dbg["maskT"][:, :, :], in_=maskT[:])

                # ---- attention per (head, query-chunk); head 0 scores done above ----
                with tc.tile_pool(name="smps", bufs=2, space="PSUM") as smps, \
                     tc.tile_pool(name="ops", bufs=2, space="PSUM") as ops:
                    for h in range(HEADS):
                        exps = exp_h0 if h == 0 else scores_and_exp(h)
                        for c in range(NQC):
                            expTm = exps[c]
                            qsl = slice(c * 512, (c + 1) * 512)
                            # mask multiply (halves)
                            for half in range(2):
                                sl = slice(half * 8, half * 8 + 8)
                                nc.vector.tensor_tensor(
                                    out=expTm[:, sl, :], in0=expTm[:, sl, :],
                                    in1=maskT[:, sl, qsl], op=OP.mult)
                            # softmax denominators: fp16 tree 16->8->4, then a
                            # 4-way accumulating ones-matmul partition reduce
                            tr = treep.tile([P, NKT // 2, 512], FP16, tag="tree")
                            nc.vector.tensor_tensor(out=tr[:], in0=expTm[:, 0:8, :],
                                                    in1=expTm[:, 8:16, :], op=OP.add)
                            nc.vector.tensor_tensor(out=tr[:, 0:4, :], in0=tr[:, 0:4, :],
                                                    in1=tr[:, 4:8, :], op=OP.add)
                            pss = smps.tile([P, 512], F32, tag="sums")
                            for t in range(4):
                                nc.tensor.matmul(pss[:], ones_h[:], tr[:, t, :],
                                                 start=(t == 0), stop=(t == 3))
                            rec = recp.tile([P, 512], F32, tag="rec")
                            nc.vector.reciprocal_approx_fast(out=rec[:], in_=pss[:])
                            # attnV -> out^T, normalize into scratch, silu -> s0T
                            po = ops.tile([P, 512], F32, tag="attnps")
                            for kt in range(NKT):
                                nc.tensor.matmul(po[:], Vg[:, kt, h * HD:(h + 1) * HD],
                                                 expTm[:, kt, :],
                                                 start=(kt == 0), stop=(kt == NKT - 1))
                            sc = scp.tile([P, 512], F32, tag="sc0")
                            nc.vector.tensor_tensor(out=sc[:], in0=po[:], in1=rec[:],
                                                    op=OP.mult)
                            nc.scalar.activation(out=s0T[:, h, qsl], in_=sc[:], func=AF.Silu)
                            if DEBUG_DUMP and h == 0 and c == 0:
                                nc.sync.dma_start(out=dbg["em00"][:, :, :], in_=expTm[:])
                    if DEBUG_DUMP:
                        nc.sync.dma_start(out=dbg["s0T"][:, :, :], in_=s0T[:])

            # ======== MLP ========
            with tc.tile_pool(name="mlp", bufs=1) as mlp, \
                 tc.tile_pool(name="p4t", bufs=4) as p4t, \
                 tc.tile_pool(name="fps", bufs=4, space="PSUM") as fps, \
                 tc.tile_pool(name="stps", bufs=2, space="PSUM") as stps:
                sq = mlp.tile([P, NEO, QLOC], FP16)
                nmu1 = mlp.tile([P, QLOC], F32R)
                sig1 = mlp.tile([P, QLOC], F32R)
                rstd1 = mlp.tile([P, QLOC], F32)
                musq = mlp.tile([P, QLOC], F32)
                var1 = mlp.tile([P, QLOC], F32)

                def t_stats(src, c, nmu, sig_o, rstd_o=None):
                    """LN stats in T layout for query chunk c of src [P, NEO, QLOC]."""
                    qsl = slice(c * 512, (c + 1) * 512)
                    nc.scalar.activation(out=sq[:, :, qsl], in_=src[:, :, qsl],
                                         func=AF.Square)
                    psm = stps.tile([P, 512], F32, tag="psmu")
                    for eo in range(NEO):
                        nc.tensor.matmul(psm[:], ones_inv[:], src[:, eo, qsl],
                                         start=(eo == 0), stop=(eo == NEO - 1))
                    pse = stps.tile([P, 512], F32, tag="pse2")
                    for eo in range(NEO):
                        nc.tensor.matmul(pse[:], ones_inv[:], sq[:, eo, qsl],
                                         start=(eo == 0), stop=(eo == NEO - 1))
                    nc.scalar.activation(out=nmu[:, qsl], in_=psm[:], func=AF.Copy,
                                         scale=-1.0)
                    nc.scalar.activation(out=musq[:, qsl], in_=psm[:], func=AF.Square)
                    nc.vector.tensor_tensor(out=var1[:, qsl], in0=pse[:],
                                            in1=musq[:, qsl], op=OP.subtract)
                    nc.scalar.activation(out=sig_o[:, qsl], in_=var1[:, qsl], func=AF.Sqrt,
                                         bias=eps_t[:], scale=1.0)
                    if rstd_o is not None:
                        nc.vector.reciprocal_approx_fast(out=rstd_o[:, qsl],
                                                         in_=sig_o[:, qsl].bitcast(F32))

                for c in range(NQC):
                    t_stats(s0T, c, nmu1, sig1, rstd1)

                # fc1 with ln1 standardization + bias folded in; swish on copyback
                h1sT = mlp.tile([P, NEO, QLOC], FP16)
                for ft in range(NEO):
                    for c in range(NQC):
                        ps = fps.tile([P, 512], F32, tag="fcps")
                        for eo in range(NEO):
                            nc.tensor.matmul(ps[:], fc1[:, eo, ft * P:(ft + 1) * P],
                                             s0T[:, eo, c * 512:(c + 1) * 512],
                                             start=(eo == 0), stop=False)
                        nc.tensor.matmul(ps[:], fc1sum[:, ft * P:(ft + 1) * P],
                                         nmu1[0:1, c * 512:(c + 1) * 512],
                                         start=False, stop=False)
                        nc.tensor.matmul(ps[:], fc1brow[:, ft * P:(ft + 1) * P],
                                         sig1[0:1, c * 512:(c + 1) * 512],
                                         start=False, stop=True)
                        nc.vector.tensor_tensor(out=ps[:], in0=ps[:],
                                                in1=rstd1[:, c * 512:(c + 1) * 512],
                                                op=OP.mult)
                        nc.scalar.activation(out=h1sT[:, ft, c * 512:(c + 1) * 512],
                                             in_=ps[:], func=AF.Silu)

                if DEBUG_DUMP:
                    nc.sync.dma_start(out=dbg["nmu1"][:, :], in_=nmu1[:].bitcast(F32))
                    nc.sync.dma_start(out=dbg["sig1"][:, :], in_=sig1[:].bitcast(F32))
                    nc.sync.dma_start(out=dbg["h1sT"][:, :, :], in_=h1sT[:])

                nmu2 = mlp.tile([P, QLOC], F32R)
                sig2 = mlp.tile([P, QLOC], F32R)
                for c in range(NQC):
                    t_stats(h1sT, c, nmu2, sig2)

                # fc2 in ROW orientation with ln2 standardization via K=1
                # augmentation. The per-row rstd2 scale is NOT applied: psum =
                # h2 * sig2_q (positive per-row scale), and the final LayerNorm
                # is invariant to per-row positive scaling.
                for qs in range(NQS):
                    ps = fps.tile([P, 512], F32, tag="fcps")
                    for eo in range(NEO):
                        nc.tensor.matmul(ps[:], h1sT[:, eo, qs * P:(qs + 1) * P],
                                         fc2[:, eo, :],
                                         start=(eo == 0), stop=False)
                    nc.tensor.matmul(ps[:], nmu2[0:1, qs * P:(qs + 1) * P],
                                     fc2sum[:, :], start=False, stop=False)
                    nc.tensor.matmul(ps[:], sig2[0:1, qs * P:(qs + 1) * P],
                                     fc2b[:, :], start=False, stop=True)
                    st6 = p4t.tile([P, 6], F32, tag="st6b")
                    nc.vector.bn_stats(out=st6[:], in_=ps[:])
                    mv = p4t.tile([P, 2], F32, tag="mvb")
                    nc.vector.bn_aggr(out=mv[:], in_=st6[:])
                    sig = p4t.tile([P, 1], F32, tag="sigb")
                    nc.scalar.activation(out=sig[:], in_=mv[:, 1:2], func=AF.Sqrt,
                                         bias=eps_t[:], scale=1.0)
                    rstd = p4t.tile([P, 1], F32, tag="rstdb")
                    nc.vector.reciprocal_approx_fast(out=rstd[:], in_=sig[:])
                    yt = p4t.tile([P, D], F32, tag="yt")
                    nc.vector.tensor_scalar(out=yt[:], in0=ps[:],
                                            scalar1=mv[:, 0:1], scalar2=rstd[:],
                                            op0=OP.subtract, op1=OP.mult)
                    if use_final_affine:
                        nc.vector.tensor_tensor(out=yt[:], in0=yt[:],
                                                in1=naff[:, 0:D], op=OP.mult)
                        nc.vector.tensor_tensor(out=yt[:], in0=yt[:],
                                                in1=naff[:, D:2 * D], op=OP.add)
                    nc.sync.dma_start(out=y_d[qs * P:(qs + 1) * P, :], in_=yt[:])

    nc.compile()
    return nc


def kernel(**inputs):
    x = np.asarray(inputs["x"], np.float32)
    conn = np.asarray(inputs["connectivity"])
    Wq = np.asarray(inputs["Wq"], np.float32)
    Wk = np.asarray(inputs["Wk"], np.float32)
    Wv = np.asarray(inputs["Wv"], np.float32)
    norm_w = np.asarray(inputs["norm_w"], np.float32)
    norm_b = np.asarray(inputs["norm_b"], np.float32)
    ln1_w = np.asarray(inputs["ln1_w"], np.float32)
    ln1_b = np.asarray(inputs["ln1_b"], np.float32)
    fc1_w = np.asarray(inputs["fc1_w"], np.float32)
    fc1_b = np.asarray(inputs["fc1_b"], np.float32)
    ln2_w = np.asarray(inputs["ln2_w"], np.float32)
    ln2_b = np.asarray(inputs["ln2_b"], np.float32)
    fc2_w = np.asarray(inputs["fc2_w"], np.float32)
    fc2_b = np.asarray(inputs["fc2_b"], np.float32)

    s = 1.0 / np.sqrt(HD)
    wq_t = np.ascontiguousarray(norm_w[:, None] * Wq.T).astype(np.float16)
    wk_t = np.ascontiguousarray((norm_w[:, None] * Wk.T) * np.float32(s)).astype(np.float16)
    wv_t = np.ascontiguousarray(norm_w[:, None] * Wv.T).astype(np.float16)
    qb = Wq @ norm_b
    kb = (Wk @ norm_b) * s
    vb = Wv @ norm_b
    fc1_t = np.ascontiguousarray(ln1_w[:, None] * fc1_w.T).astype(np.float16)
    fc1b_eff = fc1_w @ ln1_b + fc1_b
    fc2_t = np.ascontiguousarray(ln2_w[:, None] * fc2_w.T).astype(np.float16)
    fc2b_eff = fc2_w @ ln2_b + fc2_b
    # sums from the fp16-rounded matrices so the augmentation matches the matmul
    fc1sum = fc1_t.astype(np.float32).sum(axis=0)
    fc2sum = fc2_t.astype(np.float32).sum(axis=0)

    use_qk_bias = bool(np.abs(qb).max() > 0 or np.abs(kb).max() > 0)
    use_v_bias = bool(np.abs(vb).max() > 0)
    use_final_affine = not (np.allclose(norm_w, 1.0) and np.allclose(norm_b, 0.0))

    key = (use_v_bias, use_qk_bias, use_final_affine)
    if key not in _compiled:
        _compiled[key] = _build(*key)
    nc = _compiled[key]

    qkvb = np.zeros((P, 3 * NEO), np.float32)
    qkvb[:, 0:NEO] = qb.reshape(NEO, P).T
    qkvb[:, NEO:2 * NEO] = kb.reshape(NEO, P).T
    qkvb[:, 2 * NEO:3 * NEO] = vb.reshape(NEO, P).T
    naff = np.concatenate([norm_w, norm_b]).reshape(1, 2 * D).astype(np.float32)

    common = {
        "wq_t": wq_t, "wk_t": wk_t, "wv_t": wv_t,
        "fc1_t": fc1_t, "fc2_t": fc2_t,
        "fc2sum": fc2sum.reshape(1, D).astype(np.float32),
        "fc2b": fc2b_eff.reshape(1, D).astype(np.float32),
        "qkvb": qkvb, "naff": naff, "vbrow": vb.reshape(1, D).astype(np.float32),
        "onesd": np.full((P, P), 1.0 / D, np.float16),
        "fc1sum": fc1sum.reshape(1, D).astype(np.float32),
        "fc1brow": fc1b_eff.reshape(1, D).astype(np.float32),
    }

    in_maps = []
    core_ids = list(range(8))
    for c in core_ids:
        b, half = c // 2, c % 2
        qofs = half * QLOC
        xr = np.roll(x[b], -qofs, axis=0)
        cm = np.roll(np.roll(conn[b, 0], -qofs, axis=0), -qofs, axis=1)
        maskT = cm[:QLOC, :].T.astype(np.float16)            # [A, QLOC]
        maskT = np.ascontiguousarray(
            maskT.reshape(NKT, P, QLOC).transpose(1, 0, 2))          # [P, NKT, QLOC]
        in_maps.append({"x": np.ascontiguousarray(xr), "maskT": maskT, **common})

    res = run_bass_kernel_spmd(nc, in_maps, core_ids)

    y = np.empty((B, A, D), np.float32)
    for c in core_ids:
        b, half = c // 2, c % 2
        y[b, half * QLOC:(half + 1) * QLOC] = res.results[c]["y"]
    return y


# revision 19
# speedup vs baseline: 1.0329x; 1.0329x over previous
"""DotGATHead Trainium2 kernel: LN -> 4-head masked attention -> MLP (2x swish+LN+linear) -> LN.

Sharding: query rows. 8 cores = 4 batches x 2 query-halves. Each core gets its
batch's x (rolled so its query rows are always rows 0..1023 -> one SPMD program),
computes K/V over all 2048 nodes, attention + MLP for its 1024 queries.

v2 schedule: P1 (LN+transpose) and P2 (projections) fused into a 512-node chunk
pipeline so attention starts ~40us in; V projection half-deferred to fill PE
stalls during head-0 scores/exp; all 16-bit data in fp16 (10-bit mantissa =
f32r precision for these magnitudes); mask on the scalar DMA ring, x+weights on
the sync ring in consumption order; ACT tables preloaded at t=0; attention
processed in (head, 512-query-chunk) units with a 16->8->4 adder tree + 4-way
accumulating ones-matmul partition reduce; attention output normalized into a
scratch then silu'd straight into s0T (fp16) so the MLP input is ready when
attention ends. LN1/LN2 standardization folded into fc1/fc2 via K=1
augmentation matmuls; final LN via bn_stats on fc2 psum rows.
"""

import numpy as np
import ml_dtypes

import concourse.bass as bass
import concourse.mybir as mybir
import concourse.tile as tile
from concourse import bacc
from concourse.bass_utils import run_bass_kernel_spmd
from concourse.masks import make_identity

B, A, D, HEADS, HD = 4, 2048, 512, 4, 128
P = 128
QLOC = 1024              # query rows per core
NQS = QLOC // P          # 8 query subtiles
NKT = A // P             # 16 key tiles
NEO = D // P             # 4 feature partition-tiles
NQC = QLOC // 512        # 2 query chunks of 512
NCH = A // 512           # 4 node chunks of 512
EPS = 1e-5
F32 = mybir.dt.float32
F32R = mybir.dt.float32r
FP16 = mybir.dt.float16
AF = mybir.ActivationFunctionType
OP = mybir.AluOpType

_compiled = {}


DEBUG_DUMP = False


def _build(use_v_bias, use_qk_bias, use_final_affine):
    nc = bacc.Bacc("TRN2", target_bir_lowering=False, debug=False, num_devices=8)
    dbg = {}
    if DEBUG_DUMP:
        dbg["KT"] = nc.dram_tensor("d_KT", [P, HEADS, A], FP16, kind="ExternalOutput")
        dbg["QT"] = nc.dram_tensor("d_QT", [P, HEADS, QLOC], FP16, kind="ExternalOutput")
        dbg["Vg"] = nc.dram_tensor("d_Vg", [P, NKT, D], FP16, kind="ExternalOutput")
        dbg["maskT"] = nc.dram_tensor("d_maskT", [P, NKT, QLOC], FP16, kind="ExternalOutput")
        dbg["em00"] = nc.dram_tensor("d_em00", [P, NKT, 512], FP16, kind="ExternalOutput")
        dbg["s0T"] = nc.dram_tensor("d_s0T", [P, NEO, QLOC], FP16, kind="ExternalOutput")
        dbg["nmu1"] = nc.dram_tensor("d_nmu1", [P, QLOC], F32, kind="ExternalOutput")
        dbg["sig1"] = nc.dram_tensor("d_sig1", [P, QLOC], F32, kind="ExternalOutput")
        dbg["h1sT"] = nc.dram_tensor("d_h1sT", [P, NEO, QLOC], FP16, kind="ExternalOutput")

    x_d = nc.dram_tensor("x", [A, D], F32, kind="ExternalInput")
    mask_d = nc.dram_tensor("maskT", [P, NKT, QLOC], FP16, kind="ExternalInput")
    wq_d = nc.dram_tensor("wq_t", [D, D], FP16, kind="ExternalInput")
    wk_d = nc.dram_tensor("wk_t", [D, D], FP16, kind="ExternalInput")
    wv_d = nc.dram_tensor("wv_t", [D, D], FP16, kind="ExternalInput")
    fc1_d = nc.dram_tensor("fc1_t", [D, D], FP16, kind="ExternalInput")
    fc2_d = nc.dram_tensor("fc2_t", [D, D], FP16, kind="ExternalInput")
    fc2sum_d = nc.dram_tensor("fc2sum", [1, D], F32R, kind="ExternalInput")
    fc2b_d = nc.dram_tensor("fc2b", [1, D], F32R, kind="ExternalInput")
    qkvb_d = nc.dram_tensor("qkvb", [P, 3 * NEO], F32, kind="ExternalInput")
    naff_d = nc.dram_tensor("naff", [1, 2 * D], F32, kind="ExternalInput")
    vbrow_d = nc.dram_tensor("vbrow", [1, D], F32, kind="ExternalInput")
    onesd_d = nc.dram_tensor("onesd", [P, P], FP16, kind="ExternalInput")
    fc1sum_d = nc.dram_tensor("fc1sum", [1, D], F32R, kind="ExternalInput")
    fc1brow_d = nc.dram_tensor("fc1brow", [1, D], F32R, kind="ExternalInput")
    y_d = nc.dram_tensor("y", [QLOC, D], F32, kind="ExternalOutput")

    with tile.TileContext(nc) as tc:
        with tc.tile_pool(name="const", bufs=1) as const, \
             tc.tile_pool(name="mlpin", bufs=1) as mlpin:
            # --- ACT table preloads (overlap table DMA with input DMA) ---
            dummy = const.tile([P, 1], F32)
            nc.vector.memset(dummy[:], 1.0)
            for fn in (AF.Sqrt, AF.Exp, AF.Silu, AF.Square):
                nc.scalar.activation(out=dummy[:], in_=dummy[:], func=fn)
            nc.scalar.copy(out=dummy[:], in_=dummy[:])

            eps_t = const.tile([P, 1], F32)
            nc.vector.memset(eps_t[:], EPS)
            ident = const.tile([P, P], F32)
            make_identity(nc, ident[:])
            ones_h = const.tile([P, P], FP16)
            nc.vector.memset(ones_h[:], 1.0)
            ones_inv = const.tile([P, P], FP16)
            nc.sync.dma_start(out=ones_inv[:], in_=onesd_d[:, :])
            fc2sum = const.tile([1, D], F32R)
            nc.sync.dma_start(out=fc2sum[:], in_=fc2sum_d[:])
            fc2b = const.tile([1, D], F32R)
            nc.sync.dma_start(out=fc2b[:], in_=fc2b_d[:])
            fc1sum = const.tile([1, D], F32R)
            nc.sync.dma_start(out=fc1sum[:], in_=fc1sum_d[:])
            fc1brow = const.tile([1, D], F32R)
            nc.sync.dma_start(out=fc1brow[:], in_=fc1brow_d[:])
            qkvb = const.tile([P, 3 * NEO], F32)
            if use_qk_bias or use_v_bias:
                nc.sync.dma_start(out=qkvb[:], in_=qkvb_d[:])
            vb_rep = const.tile([P, D], F32)
            if use_v_bias:
                vb_ap = vbrow_d[:, :]
                nc.gpsimd.dma_start(out=vb_rep[:], in_=bass.AP(
                    tensor=vb_ap.tensor, offset=vb_ap.offset,
                    ap=[[0, P], [1, D]]))
            naff = const.tile([P, 2 * D], F32)
            if use_final_affine:
                naff_ap = naff_d[:, :]
                nc.gpsimd.dma_start(out=naff[:], in_=bass.AP(
                    tensor=naff_ap.tensor, offset=naff_ap.offset,
                    ap=[[0, P], [1, 2 * D]]))
            fc1 = const.tile([P, NEO, D], FP16)
            fc2 = const.tile([P, NEO, D], FP16)

            s0T = mlpin.tile([P, NEO, QLOC], FP16)   # silu(attn out), T layout

            # ======== attention-lifetime pools ========
            with tc.tile_pool(name="attw", bufs=1) as attw, \
                 tc.tile_pool(name="expp", bufs=2) as expp, \
                 tc.tile_pool(name="treep", bufs=2) as treep, \
                 tc.tile_pool(name="recp", bufs=2) as recp, \
                 tc.tile_pool(name="scp", bufs=2) as scp, \
                 tc.tile_pool(name="sps", bufs=2, space="PSUM") as sps:
                KT = attw.tile([P, HEADS, A], FP16)       # K^T/sqrt(hd): [d, h, node]
                QT = attw.tile([P, HEADS, QLOC], FP16)    # Q^T: [d, h, q]
                Vg = attw.tile([P, NKT, D], FP16)         # V rows [node, f]
                maskT = attw.tile([P, NKT, QLOC], FP16)

                def scores_and_exp(h):
                    """scores^T + exp for head h, both query chunks."""
                    out = []
                    for c in range(NQC):
                        expTm = expp.tile([P, NKT, 512], FP16, tag="expTm")
                        out.append(expTm)
                        for kp in range(NKT // 2):
                            ps = sps.tile([P, 2, 512], F32, tag="sc")
                            for j in range(2):
                                kt = kp * 2 + j
                                nc.tensor.matmul(ps[:, j, :],
                                                 KT[:, h, kt * P:(kt + 1) * P],
                                                 QT[:, h, c * 512:(c + 1) * 512],
                                                 start=True, stop=True)
                            nc.scalar.activation(
                                out=expTm[:, kp * 2:kp * 2 + 2, :],
                                in_=ps[:], func=AF.Exp)
                    return out

                # ---- fused P1+P2: LN chunks -> K/Q projections (+half of V) ----
                with tc.tile_pool(name="xnp", bufs=1) as xnp, \
                     tc.tile_pool(name="p1t", bufs=4) as p1t, \
                     tc.tile_pool(name="p1ps", bufs=1, space="PSUM") as p1ps, \
                     tc.tile_pool(name="p2ps", bufs=2, space="PSUM") as p2ps:
                    xnT = xnp.tile([P, NEO, A], FP16)
                    wq = xnp.tile([P, NEO, D], FP16)
                    wk = xnp.tile([P, NEO, D], FP16)
                    wv = xnp.tile([P, NEO, D], FP16)

                    def v_tile(kt):
                        ps = p2ps.tile([P, D], F32, tag="ps")
                        for eo in range(NEO):
                            nc.tensor.matmul(ps[:], xnT[:, eo, kt * P:(kt + 1) * P],
                                             wv[:, eo, :],
                                             start=(eo == 0), stop=(eo == NEO - 1))
                        if use_v_bias:
                            nc.vector.tensor_tensor(out=ps[:], in0=ps[:],
                                                    in1=vb_rep[:], op=OP.add)
                        nc.vector.tensor_copy(out=Vg[:, kt, :], in_=ps[:])

                    for c in range(NCH):
                        for j in range(4):
                            r = 4 * c + j
                            xt = p1t.tile([P, D], F32, tag="xt")
                            nc.sync.dma_start(out=xt[:], in_=x_d[r * P:(r + 1) * P, :])
                            # weight DMAs interleaved in consumption order.
                            # NOTE: in Tile, emission order IS dataflow order —
                            # every weight DMA must be emitted before its first
                            # consumer or the consumer reads uninitialized SBUF.
                            if r == 1:
                                nc.sync.dma_start(
                                    out=wk[:], in_=wk_d.rearrange("(eo p) f -> p eo f", p=P))
                            elif r == 2:
                                nc.sync.dma_start(
                                    out=wq[:], in_=wq_d.rearrange("(eo p) f -> p eo f", p=P))
                            elif r == 3:
                                nc.sync.dma_start(
                                    out=wv[:], in_=wv_d.rearrange("(eo p) f -> p eo f", p=P))
                            elif r == 6:
                                # first mask half on the scalar ring: late enough
                                # not to starve x tiles, early enough for head 0
                                nc.scalar.dma_start(out=maskT[:, 0:8, :],
                                                    in_=mask_d[:, 0:8, :])
                            elif r == 10:
                                nc.scalar.dma_start(out=maskT[:, 8:16, :],
                                                    in_=mask_d[:, 8:16, :])
                            # row LayerNorm (standardize only)
                            st6 = p1t.tile([P, 6], F32, tag="st6")
                            nc.vector.bn_stats(out=st6[:], in_=xt[:])
                            mv = p1t.tile([P, 2], F32, tag="mv")
                            nc.vector.bn_aggr(out=mv[:], in_=st6[:])
                            sig = p1t.tile([P, 1], F32, tag="sig")
                            nc.scalar.activation(out=sig[:], in_=mv[:, 1:2], func=AF.Sqrt,
                                                 bias=eps_t[:], scale=1.0)
                            rstd = p1t.tile([P, 1], F32, tag="rstd")
                            nc.vector.reciprocal_approx_fast(out=rstd[:], in_=sig[:])
                            nc.vector.tensor_scalar(out=xt[:], in0=xt[:],
                                                    scalar1=mv[:, 0:1], scalar2=rstd[:],
                                                    op0=OP.subtract, op1=OP.mult)
                            tp = p1ps.tile([P, NEO, P], F32, tag="tp1")
                            for eo in range(NEO):
                                nc.tensor.transpose(tp[:, eo, :], xt[:, eo * P:(eo + 1) * P],
                                                    ident[:])
                            nc.vector.tensor_copy(out=xnT[:, :, r * P:(r + 1) * P], in_=tp[:])
                        # K^T for this node chunk, all heads
                        for h in range(HEADS):
                            ps = p2ps.tile([P, 512], F32, tag="ps")
                            for eo in range(NEO):
                                nc.tensor.matmul(ps[:], wk[:, eo, h * HD:(h + 1) * HD],
                                                 xnT[:, eo, c * 512:(c + 1) * 512],
                                                 start=(eo == 0), stop=(eo == NEO - 1))
                            if use_qk_bias:
                                nc.scalar.activation(out=KT[:, h, c * 512:(c + 1) * 512],
                                                     in_=ps[:], func=AF.Identity,
                                                     bias=qkvb[:, NEO + h:NEO + h + 1],
                                                     scale=1.0)
                            else:
                                nc.scalar.copy(out=KT[:, h, c * 512:(c + 1) * 512], in_=ps[:])
                        # Q^T (queries = rolled nodes 0..QLOC-1)
                        if c < NQC:
                            for h in range(HEADS):
                                ps = p2ps.tile([P, 512], F32, tag="ps")
                                for eo in range(NEO):
                                    nc.tensor.matmul(ps[:], wq[:, eo, h * HD:(h + 1) * HD],
                                                     xnT[:, eo, c * 512:(c + 1) * 512],
                                                     start=(eo == 0), stop=(eo == NEO - 1))
                                if use_qk_bias:
                                    nc.scalar.activation(out=QT[:, h, c * 512:(c + 1) * 512],
                                                         in_=ps[:], func=AF.Identity,
                                                         bias=qkvb[:, h:h + 1], scale=1.0)
                                else:
                                    nc.vector.tensor_copy(out=QT[:, h, c * 512:(c + 1) * 512],
                                                          in_=ps[:])
                        # half of V in-chunk (kt = 4c, 4c+1)
                        v_tile(4 * c)
                        v_tile(4 * c + 1)

                    # head-0 scores+exp next in priority; deferred V fills PE stalls
                    exp_h0 = scores_and_exp(0)
                    nc.sync.dma_start(
                        out=fc1[:], in_=fc1_d.rearrange("(eo p) f -> p eo f", p=P))
                    nc.sync.dma_start(
                        out=fc2[:], in_=fc2_d.rearrange("(eo p) f -> p eo f", p=P))
                    for c in range(NCH):
                        v_tile(4 * c + 2)
                        v_tile(4 * c + 3)
                    if DEBUG_DUMP:
                        nc.sync.dma_start(out=dbg["KT"][:, :, :], in_=KT[:])
                        nc.sync.dma_start(out=dbg["QT"][:, :, :], in_=QT[:])
                        nc.sync.dma_start(out=dbg["Vg"][:, :, :], in_=Vg[:])
                        nc.sync.dma_start(out=dbg["maskT"][:, :, :], in_=maskT[:])

                # ---- attention per (head, query-chunk); head 0 scores done above ----
                with tc.tile_pool(name="smps", bufs=2, space="PSUM") as smps, \
                     tc.tile_pool(name="ops", bufs=2, space="PSUM") as ops:
                    for h in range(HEADS):
                        exps = exp_h0 if h == 0 else scores_and_exp(h)
                        for c in range(NQC):
                            expTm = exps[c]
                            qsl = slice(c * 512, (c + 1) * 512)
                            # mask multiply (halves)
                            for half in range(2):
                                sl = slice(half * 8, half * 8 + 8)
                                nc.vector.tensor_tensor(
                                    out=expTm[:, sl, :], in0=expTm[:, sl, :],
                                    in1=maskT[:, sl, qsl], op=OP.mult)
                            # softmax denominators: fp16 tree 16->8->4, then a
                            # 4-way accumulating ones-matmul partition reduce
                            tr = treep.tile([P, NKT // 2, 512], FP16, tag="tree")
                            nc.vector.tensor_tensor(out=tr[:], in0=expTm[:, 0:8, :],
                                                    in1=expTm[:, 8:16, :], op=OP.add)
                            nc.vector.tensor_tensor(out=tr[:, 0:4, :], in0=tr[:, 0:4, :],
                                                    in1=tr[:, 4:8, :], op=OP.add)
                            pss = smps.tile([P, 512], F32, tag="sums")
                            for t in range(4):
                                nc.tensor.matmul(pss[:], ones_h[:], tr[:, t, :],
                                                 start=(t == 0), stop=(t == 3))
                            rec = recp.tile([P, 512], F32, tag="rec")
                            nc.vector.reciprocal_approx_fast(out=rec[:], in_=pss[:])
                            # attnV -> out^T, normalize into scratch, silu -> s0T
                            po = ops.tile([P, 512], F32, tag="attnps")
                            for kt in range(NKT):
                                nc.tensor.matmul(po[:], Vg[:, kt, h * HD:(h + 1) * HD],
                                                 expTm[:, kt, :],
                                                 start=(kt == 0), stop=(kt == NKT - 1))
                            sc = scp.tile([P, 512], F32, tag="sc0")
                            nc.vector.tensor_tensor(out=sc[:], in0=po[:], in1=rec[:],
                                                    op=OP.mult)
                            nc.scalar.activation(out=s0T[:, h, qsl], in_=sc[:], func=AF.Silu)
                            if DEBUG_DUMP and h == 0 and c == 0:
                                nc.sync.dma_start(out=dbg["em00"][:, :, :], in_=expTm[:])
                    if DEBUG_DUMP:
                        nc.sync.dma_start(out=dbg["s0T"][:, :, :], in_=s0T[:])

            # ======== MLP ========
            with tc.tile_pool(name="mlp", bufs=1) as mlp, \
                 tc.tile_pool(name="p4t", bufs=4) as p4t, \
                 tc.tile_pool(name="fps", bufs=4, space="PSUM") as fps, \
                 tc.tile_pool(name="stps", bufs=2, space="PSUM") as stps:
                sq = mlp.tile([P, NEO, QLOC], FP16)
                nmu1 = mlp.tile([P, QLOC], F32R)
                sig1 = mlp.tile([P, QLOC], F32R)
                rstd1 = mlp.tile([P, QLOC], F32)
                musq = mlp.tile([P, QLOC], F32)
                var1 = mlp.tile([P, QLOC], F32)

                def t_stats(src, c, nmu, sig_o, rstd_o=None):
                    """LN stats in T layout for query chunk c of src [P, NEO, QLOC]."""
                    qsl = slice(c * 512, (c + 1) * 512)
                    # square on DVE (fp16 TT 2x) to keep ACT free for exp/silu
                    nc.vector.tensor_tensor(out=sq[:, :, qsl], in0=src[:, :, qsl],
                                            in1=src[:, :, qsl], op=OP.mult)
                    psm = stps.tile([P, 512], F32, tag="psmu")
                    for eo in range(NEO):
                        nc.tensor.matmul(psm[:], ones_inv[:], src[:, eo, qsl],
                                         start=(eo == 0), stop=(eo == NEO - 1))
                    pse = stps.tile([P, 512], F32, tag="pse2")
                    for eo in range(NEO):
                        nc.tensor.matmul(pse[:], ones_inv[:], sq[:, eo, qsl],
                                         start=(eo == 0), stop=(eo == NEO - 1))
                    nc.scalar.activation(out=nmu[:, qsl], in_=psm[:], func=AF.Copy,
                                         scale=-1.0)
                    nmu32 = nmu[:, qsl].bitcast(F32)
                    nc.vector.tensor_tensor(out=musq[:, qsl], in0=nmu32,
                                            in1=nmu32, op=OP.mult)
                    nc.vector.tensor_tensor(out=var1[:, qsl], in0=pse[:],
                                            in1=musq[:, qsl], op=OP.subtract)
                    nc.scalar.activation(out=sig_o[:, qsl], in_=var1[:, qsl], func=AF.Sqrt,
                                         bias=eps_t[:], scale=1.0)
                    if rstd_o is not None:
                        nc.vector.reciprocal_approx_fast(out=rstd_o[:, qsl],
                                                         in_=sig_o[:, qsl].bitcast(F32))

                for c in range(NQC):
                    t_stats(s0T, c, nmu1, sig1, rstd1)

                # fc1 with ln1 standardization + bias folded in; swish on copyback
                h1sT = mlp.tile([P, NEO, QLOC], FP16)
                for ft in range(NEO):
                    for c in range(NQC):
                        ps = fps.tile([P, 512], F32, tag="fcps")
                        for eo in range(NEO):
                            nc.tensor.matmul(ps[:], fc1[:, eo, ft * P:(ft + 1) * P],
                                             s0T[:, eo, c * 512:(c + 1) * 512],
                                             start=(eo == 0), stop=False)
                        nc.tensor.matmul(ps[:], fc1sum[:, ft * P:(ft + 1) * P],
                                         nmu1[0:1, c * 512:(c + 1) * 512],
                                         start=False, stop=False)
                        nc.tensor.matmul(ps[:], fc1brow[:, ft * P:(ft + 1) * P],
                                         sig1[0:1, c * 512:(c + 1) * 512],
                                         start=False, stop=True)
                        nc.vector.tensor_tensor(out=ps[:], in0=ps[:],
                                                in1=rstd1[:, c * 512:(c + 1) * 512],
                                                op=OP.mult)
                        nc.scalar.activation(out=h1sT[:, ft, c * 512:(c + 1) * 512],
                                             in_=ps[:], func=AF.Silu)

                if DEBUG_DUMP:
                    nc.sync.dma_start(out=dbg["nmu1"][:, :], in_=nmu1[:].bitcast(F32))
                    nc.sync.dma_start(out=dbg["sig1"][:, :], in_=sig1[:].bitcast(F32))
                    nc.sync.dma_start(out=dbg["h1sT"][:, :, :], in_=h1sT[:])

                nmu2 = mlp.tile([P, QLOC], F32R)
                sig2 = mlp.tile([P, QLOC], F32R)
                for c in range(NQC):
                    t_stats(h1sT, c, nmu2, sig2)

                # fc2 in ROW orientation with ln2 standardization via K=1
                # augmentation. The per-row rstd2 scale is NOT applied: psum =
                # h2 * sig2_q (positive per-row scale), and the final LayerNorm
                # is invariant to per-row positive scaling.
                for qs in range(NQS):
                    ps = fps.tile([P, 512], F32, tag="fcps")
                    for eo in range(NEO):
                        nc.tensor.matmul(ps[:], h1sT[:, eo, qs * P:(qs + 1) * P],
                                         fc2[:, eo, :],
                                         start=(eo == 0), stop=False)
                    nc.tensor.matmul(ps[:], nmu2[0:1, qs * P:(qs + 1) * P],
                                     fc2sum[:, :], start=False, stop=False)
                    nc.tensor.matmul(ps[:], sig2[0:1, qs * P:(qs + 1) * P],
                                     fc2b[:, :], start=False, stop=True)
                    st6 = p4t.tile([P, 6], F32, tag="st6b")
                    nc.vector.bn_stats(out=st6[:], in_=ps[:])
                    mv = p4t.tile([P, 2], F32, tag="mvb")
                    nc.vector.bn_aggr(out=mv[:], in_=st6[:])
                    sig = p4t.tile([P, 1], F32, tag="sigb")
                    nc.scalar.activation(out=sig[:], in_=mv[:, 1:2], func=AF.Sqrt,
                                         bias=eps_t[:], scale=1.0)
                    rstd = p4t.tile([P, 1], F32, tag="rstdb")
                    nc.vector.reciprocal_approx_fast(out=rstd[:], in_=sig[:])
                    yt = p4t.tile([P, D], F32, tag="yt")
                    nc.vector.tensor_scalar(out=yt[:], in0=ps[:],
                                            scalar1=mv[:, 0:1], scalar2=rstd[:],
                                            op0=OP.subtract, op1=OP.mult)
                    if use_final_affine:
                        nc.vector.tensor_tensor(out=yt[:], in0=yt[:],
                                                in1=naff[:, 0:D], op=OP.mult)
                        nc.vector.tensor_tensor(out=yt[:], in0=yt[:],
                                                in1=naff[:, D:2 * D], op=OP.add)
                    nc.sync.dma_start(out=y_d[qs * P:(qs + 1) * P, :], in_=yt[:])

    nc.compile()
    return nc


def kernel(**inputs):
    x = np.asarray(inputs["x"], np.float32)
    conn = np.asarray(inputs["connectivity"])
    Wq = np.asarray(inputs["Wq"], np.float32)
    Wk = np.asarray(inputs["Wk"], np.float32)
    Wv = np.asarray(inputs["Wv"], np.float32)
    norm_w = np.asarray(inputs["norm_w"], np.float32)
    norm_b = np.asarray(inputs["norm_b"], np.float32)
    ln1_w = np.asarray(inputs["ln1_w"], np.float32)
    ln1_b = np.asarray(inputs["ln1_b"], np.float32)
    fc1_w = np.asarray(inputs["fc1_w"], np.float32)
    fc1_b = np.asarray(inputs["fc1_b"], np.float32)
    ln2_w = np.asarray(inputs["ln2_w"], np.float32)
    ln2_b = np.asarray(inputs["ln2_b"], np.float32)
    fc2_w = np.asarray(inputs["fc2_w"], np.float32)
    fc2_b = np.asarray(inputs["fc2_b"], np.float32)

    s = 1.0 / np.sqrt(HD)
    wq_t = np.ascontiguousarray(norm_w[:, None] * Wq.T).astype(np.float16)
    wk_t = np.ascontiguousarray((norm_w[:, None] * Wk.T) * np.float32(s)).astype(np.float16)
    wv_t = np.ascontiguousarray(norm_w[:, None] * Wv.T).astype(np.float16)
    qb = Wq @ norm_b
    kb = (Wk @ norm_b) * s
    vb = Wv @ norm_b
    fc1_t = np.ascontiguousarray(ln1_w[:, None] * fc1_w.T).astype(np.float16)
    fc1b_eff = fc1_w @ ln1_b + fc1_b
    fc2_t = np.ascontiguousarray(ln2_w[:, None] * fc2_w.T).astype(np.float16)
    fc2b_eff = fc2_w @ ln2_b + fc2_b
    # sums from the fp16-rounded matrices so the augmentation matches the matmul
    fc1sum = fc1_t.astype(np.float32).sum(axis=0)
    fc2sum = fc2_t.astype(np.float32).sum(axis=0)

    use_qk_bias = bool(np.abs(qb).max() > 0 or np.abs(kb).max() > 0)
    use_v_bias = bool(np.abs(vb).max() > 0)
    use_final_affine = not (np.allclose(norm_w, 1.0) and np.allclose(norm_b, 0.0))

    key = (use_v_bias, use_qk_bias, use_final_affine)
    if key not in _compiled:
        _compiled[key] = _build(*key)
    nc = _compiled[key]

    qkvb = np.zeros((P, 3 * NEO), np.float32)
    qkvb[:, 0:NEO] = qb.reshape(NEO, P).T
    qkvb[:, NEO:2 * NEO] = kb.reshape(NEO, P).T
    qkvb[:, 2 * NEO:3 * NEO] = vb.reshape(NEO, P).T
    naff = np.concatenate([norm_w, norm_b]).reshape(1, 2 * D).astype(np.float32)

    common = {
        "wq_t": wq_t, "wk_t": wk_t, "wv_t": wv_t,
        "fc1_t": fc1_t, "fc2_t": fc2_t,
        "fc2sum": fc2sum.reshape(1, D).astype(np.float32),
        "fc2b": fc2b_eff.reshape(1, D).astype(np.float32),
        "qkvb": qkvb, "naff": naff, "vbrow": vb.reshape(1, D).astype(np.float32),
        "onesd": np.full((P, P), 1.0 / D, np.float16),
        "fc1sum": fc1sum.reshape(1, D).astype(np.float32),
        "fc1brow": fc1b_eff.reshape(1, D).astype(np.float32),
    }

    in_maps = []
    core_ids = list(range(8))
    for c in core_ids:
        b, half = c // 2, c % 2
        qofs = half * QLOC
        xr = np.roll(x[b], -qofs, axis=0)
        cm = np.roll(np.roll(conn[b, 0], -qofs, axis=0), -qofs, axis=1)
        maskT = cm[:QLOC, :].T.astype(np.float16)            # [A, QLOC]
        maskT = np.ascontiguousarray(
            maskT.reshape(NKT, P, QLOC).transpose(1, 0, 2))          # [P, NKT, QLOC]
        in_maps.append({"x": np.ascontiguousarray(xr), "maskT": maskT, **common})

    res = run_bass_kernel_spmd(nc, in_maps, core_ids)

    y = np.empty((B, A, D), np.float32)
    for c in core_ids:
        b, half = c // 2, c % 2
        y[b, half * QLOC:(half + 1) * QLOC] = res.results[c]["y"]
    return y
